# revision 36
# baseline (speedup 1.0000x reference)
"""Trainium2 Bass kernel for nn_DiscreteDiffusion (q_sample with logits).

Math: for each batch b with diffusion step t[b]:
    alpha = alpha_cumprod[t[b]]
    beta  = (1 - alpha) / C              (uniform-target mass per class)
    q_small[c,d] = alpha * I[c,d] + beta  (all off-diagonal entries equal beta,
                                           diagonal entries equal alpha + beta)
    log_q = log(q_small + EPS) has only TWO distinct values:
        v0 = log(beta + EPS)            (off-diagonal)
        v1 = log(alpha + beta + EPS)    (diagonal)
    =>  logits[s,d] = (v1 - v0) * x0[s,d] + v0 * sum_c x0[s,c]
    q_t[s] = q_small for every position s  (pure broadcast -> memory bound)

Sharding: pure data parallel over batch, 8 batches per core on 8 cores.
"""

import numpy as np
from contextlib import ExitStack

import concourse.bass as bass
import concourse.bacc as bacc
import concourse.tile as tile
from concourse import mybir
from concourse.bass_utils import run_bass_kernel_spmd

F32 = mybir.dt.float32
ALU = mybir.AluOpType
ACTF = mybir.ActivationFunctionType

T_STEPS = 200
C = 21
EPS = 1e-10
PRECISION = 1e-4

N_CORES = 8
B = 64
S = 4096
B_LOC = B // N_CORES          # 8 batches per core

P = 128                       # SBUF partitions
SPP = S // P                  # 32 sequence rows per partition (per batch)
FREE = SPP * C                # 672 floats per partition per batch
CC = C * C                    # 441
QT_REP = 2                    # DMA replication factor (step-0 source dim)
QT_F = (SPP // QT_REP) * CC   # floats per partition per qt tile
assert P * QT_REP * QT_F == S * CC
# single packed small-input row: t | iota | alpha table
# (one DMA so consuming DVE ops carry a single sync wait — the STT
#  instruction struct has only one wait slot)
CST_N = B_LOC + 2 * T_STEPS


def _alpha_table() -> np.ndarray:
    """fp32 replica of reference._alpha_cumprod (cos computed in f64, rounded)."""
    steps = np.arange(T_STEPS + 1, dtype=np.float32) / np.float32(T_STEPS)
    arg = (steps + np.float32(PRECISION)) / np.float32(1.0 + PRECISION)
    arg = arg * np.float32(np.pi) / np.float32(2.0)
    ac = np.cos(arg.astype(np.float64)).astype(np.float32) ** 2
    ac = ac / ac[0]
    betas = np.clip((np.float32(1.0) - ac[1:] / ac[:-1]), 0.0, 0.999).astype(np.float32)
    alphas = np.float32(1.0) - betas
    return np.cumprod(alphas, dtype=np.float32)  # [200]


def _bcast_p(ap: bass.AP, p: int = P) -> bass.AP:
    """[1, N] DRAM access pattern -> [p, N] partition-broadcast (step 0)."""
    return bass.AP(tensor=ap.tensor, offset=ap.offset, ap=[[0, p]] + list(ap.ap[1:]))


def emit_kernel(tc, aps: dict):
    """Emit the per-core program. aps maps names -> DRAM APs."""
    nc = tc.nc
    x0_d = aps["x0"]          # [B_LOC, S, C] f32 in
    cst_d = aps["cst"]        # [1, CST_N] f32 in (t | iota | table)
    logits_d = aps["logits"]  # [B_LOC, S, C] f32 out
    qt_d = aps["qt"]          # [B_LOC, P, QT_REP, QT_F] f32 out

    with ExitStack() as ctx:
        singles = ctx.enter_context(tc.tile_pool(name="singles", bufs=1))
        scratch = ctx.enter_context(tc.tile_pool(name="scratch", bufs=2))
        # Several instruction structs here (TensorScalarPtr, direct DMA)
        # have a single HW sync-wait slot, so no instruction may need two
        # semaphore waits: HWDGE carries cst + 7 q_t stores (8 lanes, no
        # reuse), the last q_t store and the x0/logits DMAs ride SWDGE,
        # and the 4-deep qt pool gives each tensor_scalar at most one
        # slot-recycling wait.
        qtp = ctx.enter_context(tc.tile_pool(name="qtp", bufs=4))

        # ---- one broadcast DMA for every small input ---------------------
        cst_sb = singles.tile([P, CST_N], F32)
        nc.gpsimd.dma_start(out=cst_sb, in_=_bcast_p(cst_d))
        o = B_LOC
        t_sb = cst_sb[:, 0:B_LOC]
        idx_sb = cst_sb[:, o : o + T_STEPS]
        tab_sb = cst_sb[:, o + T_STEPS : o + 2 * T_STEPS]

        # diag mask built on device (no DMA): 0s via gpsimd, 1s on the
        # 441-block diagonals via a strided DVE memset
        n_rep_cc = QT_F // CC
        msk_sb = singles.tile([P, QT_F], F32)
        nc.gpsimd.memset(msk_sb, 0.0)
        ma = msk_sb[:, :]
        m_diag = bass.AP(
            tensor=ma.tensor, offset=ma.offset,
            ap=[list(ma.ap[0]), [CC, n_rep_cc], [C + 1, C]],
        )
        nc.vector.memset(m_diag, 1.0)

        # ---- per-batch scalars, computed redundantly on all partitions ---
        alpha = singles.tile([P, B_LOC], F32)
        beta = singles.tile([P, B_LOC], F32)
        diag = singles.tile([P, B_LOC], F32)
        v0 = singles.tile([P, B_LOC], F32)
        v1 = singles.tile([P, B_LOC], F32)
        v10 = singles.tile([P, B_LOC], F32)

        for b in range(B_LOC):
            eq = scratch.tile([P, T_STEPS], F32, tag="eq")
            # eq = (idx == t[b]) * table ; alpha[b] = sum(eq) (fused accum)
            nc.vector.scalar_tensor_tensor(
                out=eq, in0=idx_sb, scalar=t_sb[:, b : b + 1], in1=tab_sb,
                op0=ALU.is_equal, op1=ALU.mult,
                accum_out=alpha[:, b : b + 1],
            )
        # beta = (1 - alpha)/C  computed as alpha*(-1/C) + 1/C
        nc.vector.tensor_scalar(
            out=beta, in0=alpha, scalar1=-1.0 / C, scalar2=1.0 / C,
            op0=ALU.mult, op1=ALU.add,
        )
        nc.vector.tensor_add(diag, alpha, beta)          # alpha + beta
        eps_sb = singles.tile([P, 1], F32)
        nc.vector.memset(eps_sb, EPS)
        nc.scalar.activation(v0, beta, ACTF.Ln, bias=eps_sb[:, 0:1])  # log(beta+eps)
        nc.scalar.activation(v1, diag, ACTF.Ln, bias=eps_sb[:, 0:1])  # log(diag+eps)
        nc.vector.tensor_sub(v10, v1, v0)

        # ---- q_t: build one [P, QT_F] pattern per batch (mask*alpha+beta),
        # fan out via a DMA with a step-0 repeat dim --------------------
        for b in range(B_LOC):
            q = qtp.tile([P, QT_F], F32, tag="qt")
            nc.vector.tensor_scalar(
                out=q, in0=msk_sb,
                scalar1=alpha[:, b : b + 1], scalar2=beta[:, b : b + 1],
                op0=ALU.mult, op1=ALU.add,
            )
            qa = q[:, :]
            rep = bass.AP(
                tensor=qa.tensor, offset=qa.offset,
                ap=[list(qa.ap[0]), [0, QT_REP], list(qa.ap[1])],
            )
            nc.sync.dma_start(out=qt_d[b], in_=rep)

        # ---- logits ------------------------------------------------------
        x0_sb = singles.tile([P, B_LOC, FREE], F32)
        nc.gpsimd.dma_start(
            out=x0_sb,
            in_=x0_d.flatten().rearrange("(b p f) -> p b f", b=B_LOC, p=P, f=FREE),
        )
        r = singles.tile([P, B_LOC, SPP], F32)
        nc.vector.reduce_sum(
            out=r,
            in_=x0_sb[:, :, :].rearrange("p b (j c) -> p b j c", c=C),
            axis=mybir.AxisListType.X,
        )
        rv0 = singles.tile([P, B_LOC, SPP], F32)
        logits_sb = singles.tile([P, B_LOC, FREE], F32)
        for b in range(B_LOC):
            nc.vector.tensor_scalar(
                out=rv0[:, b, :], in0=r[:, b, :],
                scalar1=v0[:, b : b + 1], scalar2=None, op0=ALU.mult,
            )
            rva = rv0[:, b, :]
            rv_b = bass.AP(
                tensor=rva.tensor, offset=rva.offset,
                ap=[list(rva.ap[0]), list(rva.ap[1]), [0, C]],
            )
            # logits = x0 * v10 + rv0  (rv0 broadcast over the class dim)
            nc.vector.scalar_tensor_tensor(
                out=logits_sb[:, b, :].rearrange("p (j c) -> p j c", c=C),
                in0=x0_sb[:, b, :].rearrange("p (j c) -> p j c", c=C),
                scalar=v10[:, b : b + 1],
                in1=rv_b,
                op0=ALU.mult, op1=ALU.add,
            )
        nc.gpsimd.dma_start(
            out=logits_d.flatten().rearrange("(b p f) -> p b f", b=B_LOC, p=P, f=FREE),
            in_=logits_sb,
        )


def build_nc():
    nc = bacc.Bacc("TRN2", target_bir_lowering=False, debug=False, enable_asserts=False)
    aps = {
        "x0": nc.dram_tensor("x0", [B_LOC, S, C], F32, kind="ExternalInput").ap(),
        "cst": nc.dram_tensor("cst", [1, CST_N], F32, kind="ExternalInput").ap(),
        "logits": nc.dram_tensor(
            "logits", [B_LOC, S, C], F32, kind="ExternalOutput"
        ).ap(),
        "qt": nc.dram_tensor(
            "qt", [B_LOC, P, QT_REP, QT_F], F32, kind="ExternalOutput"
        ).ap(),
    }
    with tile.TileContext(nc) as tc:
        emit_kernel(tc, aps)
    nc.compile()
    return nc


_NC_CACHE = None


def _get_nc():
    global _NC_CACHE
    if _NC_CACHE is None:
        _NC_CACHE = build_nc()
    return _NC_CACHE


def make_in_maps(x_0: np.ndarray, t: np.ndarray) -> list:
    x_0 = np.ascontiguousarray(np.asarray(x_0), dtype=np.float32)
    t_f = np.asarray(t).astype(np.float32)
    tail = np.concatenate([np.arange(T_STEPS, dtype=np.float32), _alpha_table()])
    in_maps = []
    for c in range(N_CORES):
        lo = c * B_LOC
        cst = np.concatenate([t_f[lo : lo + B_LOC], tail]).reshape(1, CST_N)
        in_maps.append(
            {
                "x0": np.ascontiguousarray(x_0[lo : lo + B_LOC]),
                "cst": np.ascontiguousarray(cst),
            }
        )
    return in_maps


def assemble(results: list):
    logits = np.concatenate([r["logits"] for r in results], axis=0)
    q_t = np.concatenate(
        [r["qt"].reshape(B_LOC, S, C, C) for r in results], axis=0
    )
    return logits, q_t


def kernel(x_0, t):
    nc = _get_nc()
    res = run_bass_kernel_spmd(nc, make_in_maps(x_0, t), core_ids=list(range(N_CORES)))
    return assemble(res.results)


# revision 38
# speedup vs baseline: 1.1111x; 1.1111x over previous
"""Trainium2 Bass kernel for nn_DiscreteDiffusion (q_sample with logits).

Math: for each batch b with diffusion step t[b]:
    alpha = alpha_cumprod[t[b]]
    beta  = (1 - alpha) / C              (uniform-target mass per class)
    q_small[c,d] = alpha * I[c,d] + beta  (all off-diagonal entries equal beta,
                                           diagonal entries equal alpha + beta)
    log_q = log(q_small + EPS) has only TWO distinct values:
        v0 = log(beta + EPS)            (off-diagonal)
        v1 = log(alpha + beta + EPS)    (diagonal)
    =>  logits[s,d] = (v1 - v0) * x0[s,d] + v0 * sum_c x0[s,c]
    q_t[s] = q_small for every position s  (pure broadcast -> memory bound)

Sharding: pure data parallel over batch, 8 batches per core on 8 cores.
"""

import numpy as np
from contextlib import ExitStack

import concourse.bass as bass
import concourse.bacc as bacc
import concourse.tile as tile
from concourse import mybir
from concourse.bass_utils import run_bass_kernel_spmd

F32 = mybir.dt.float32
ALU = mybir.AluOpType
ACTF = mybir.ActivationFunctionType

T_STEPS = 200
C = 21
EPS = 1e-10
PRECISION = 1e-4

N_CORES = 8
B = 64
S = 4096
B_LOC = B // N_CORES          # 8 batches per core

P = 128                       # SBUF partitions
SPP = S // P                  # 32 sequence rows per partition (per batch)
FREE = SPP * C                # 672 floats per partition per batch
CC = C * C                    # 441
QT_REP = 4                    # DMA replication factor (step-0 source dim)
QT_F = (SPP // QT_REP) * CC   # floats per partition per qt tile
assert P * QT_REP * QT_F == S * CC
# single packed small-input row: t | iota | alpha table
# (one DMA so consuming DVE ops carry a single sync wait — the STT
#  instruction struct has only one wait slot)
CST_N = B_LOC + 2 * T_STEPS


def _alpha_table() -> np.ndarray:
    """fp32 replica of reference._alpha_cumprod (cos computed in f64, rounded)."""
    steps = np.arange(T_STEPS + 1, dtype=np.float32) / np.float32(T_STEPS)
    arg = (steps + np.float32(PRECISION)) / np.float32(1.0 + PRECISION)
    arg = arg * np.float32(np.pi) / np.float32(2.0)
    ac = np.cos(arg.astype(np.float64)).astype(np.float32) ** 2
    ac = ac / ac[0]
    betas = np.clip((np.float32(1.0) - ac[1:] / ac[:-1]), 0.0, 0.999).astype(np.float32)
    alphas = np.float32(1.0) - betas
    return np.cumprod(alphas, dtype=np.float32)  # [200]


def _bcast_p(ap: bass.AP, p: int = P) -> bass.AP:
    """[1, N] DRAM access pattern -> [p, N] partition-broadcast (step 0)."""
    return bass.AP(tensor=ap.tensor, offset=ap.offset, ap=[[0, p]] + list(ap.ap[1:]))


def emit_kernel(tc, aps: dict):
    """Emit the per-core program. aps maps names -> DRAM APs."""
    nc = tc.nc
    x0_d = aps["x0"]          # [B_LOC, S, C] f32 in
    cst_d = aps["cst"]        # [1, CST_N] f32 in (t | iota | table)
    logits_d = aps["logits"]  # [B_LOC, S, C] f32 out
    qt_d = aps["qt"]          # [B_LOC, P, QT_REP, QT_F] f32 out

    with ExitStack() as ctx:
        singles = ctx.enter_context(tc.tile_pool(name="singles", bufs=1))
        scratch = ctx.enter_context(tc.tile_pool(name="scratch", bufs=2))
        # One buffer per batch, never recycled: several instruction structs
        # here (TensorScalarPtr, direct DMA) have a single HW sync-wait
        # slot, so slot-reuse WAR waits must not exist; a deep pool also
        # lets all 8 q_t stores queue up front so the SDMA engines stay
        # saturated to the end.  The 8 q_t stores go on HWDGE (exactly its
        # 8 sem lanes, no lane-reuse waits); the other DMAs ride SWDGE.
        qtp = ctx.enter_context(tc.tile_pool(name="qtp", bufs=B_LOC))

        # ---- one broadcast DMA for every small input ---------------------
        cst_sb = singles.tile([P, CST_N], F32)
        nc.gpsimd.dma_start(out=cst_sb, in_=_bcast_p(cst_d))
        o = B_LOC
        t_sb = cst_sb[:, 0:B_LOC]
        idx_sb = cst_sb[:, o : o + T_STEPS]
        tab_sb = cst_sb[:, o + T_STEPS : o + 2 * T_STEPS]

        # diag mask built on device (no DMA): 0s via gpsimd, 1s on the
        # 441-block diagonals via a strided DVE memset
        n_rep_cc = QT_F // CC
        msk_sb = singles.tile([P, QT_F], F32)
        nc.gpsimd.memset(msk_sb, 0.0)
        ma = msk_sb[:, :]
        m_diag = bass.AP(
            tensor=ma.tensor, offset=ma.offset,
            ap=[list(ma.ap[0]), [CC, n_rep_cc], [C + 1, C]],
        )
        nc.vector.memset(m_diag, 1.0)

        # ---- per-batch scalars, computed redundantly on all partitions ---
        alpha = singles.tile([P, B_LOC], F32)
        beta = singles.tile([P, B_LOC], F32)
        diag = singles.tile([P, B_LOC], F32)
        v0 = singles.tile([P, B_LOC], F32)
        v1 = singles.tile([P, B_LOC], F32)
        v10 = singles.tile([P, B_LOC], F32)

        for b in range(B_LOC):
            eq = scratch.tile([P, T_STEPS], F32, tag="eq")
            # eq = (idx == t[b]) * table ; alpha[b] = sum(eq) (fused accum)
            nc.vector.scalar_tensor_tensor(
                out=eq, in0=idx_sb, scalar=t_sb[:, b : b + 1], in1=tab_sb,
                op0=ALU.is_equal, op1=ALU.mult,
                accum_out=alpha[:, b : b + 1],
            )
        # beta = (1 - alpha)/C  computed as alpha*(-1/C) + 1/C
        nc.vector.tensor_scalar(
            out=beta, in0=alpha, scalar1=-1.0 / C, scalar2=1.0 / C,
            op0=ALU.mult, op1=ALU.add,
        )
        nc.vector.tensor_add(diag, alpha, beta)          # alpha + beta
        eps_sb = singles.tile([P, 1], F32)
        nc.vector.memset(eps_sb, EPS)
        nc.scalar.activation(v0, beta, ACTF.Ln, bias=eps_sb[:, 0:1])  # log(beta+eps)
        nc.scalar.activation(v1, diag, ACTF.Ln, bias=eps_sb[:, 0:1])  # log(diag+eps)
        nc.vector.tensor_sub(v10, v1, v0)

        # ---- q_t: build one [P, QT_F] pattern per batch (mask*alpha+beta),
        # fan out via a DMA with a step-0 repeat dim --------------------
        for b in range(B_LOC):
            q = qtp.tile([P, QT_F], F32, tag="qt")
            nc.vector.tensor_scalar(
                out=q, in0=msk_sb,
                scalar1=alpha[:, b : b + 1], scalar2=beta[:, b : b + 1],
                op0=ALU.mult, op1=ALU.add,
            )
            qa = q[:, :]
            rep = bass.AP(
                tensor=qa.tensor, offset=qa.offset,
                ap=[list(qa.ap[0]), [0, QT_REP], list(qa.ap[1])],
            )
            nc.sync.dma_start(out=qt_d[b], in_=rep)

        # ---- logits ------------------------------------------------------
        x0_sb = singles.tile([P, B_LOC, FREE], F32)
        nc.gpsimd.dma_start(
            out=x0_sb,
            in_=x0_d.flatten().rearrange("(b p f) -> p b f", b=B_LOC, p=P, f=FREE),
        )
        r = singles.tile([P, B_LOC, SPP], F32)
        nc.vector.reduce_sum(
            out=r,
            in_=x0_sb[:, :, :].rearrange("p b (j c) -> p b j c", c=C),
            axis=mybir.AxisListType.X,
        )
        rv0 = singles.tile([P, B_LOC, SPP], F32)
        logits_sb = singles.tile([P, B_LOC, FREE], F32)
        for b in range(B_LOC):
            nc.vector.tensor_scalar(
                out=rv0[:, b, :], in0=r[:, b, :],
                scalar1=v0[:, b : b + 1], scalar2=None, op0=ALU.mult,
            )
            rva = rv0[:, b, :]
            rv_b = bass.AP(
                tensor=rva.tensor, offset=rva.offset,
                ap=[list(rva.ap[0]), list(rva.ap[1]), [0, C]],
            )
            # logits = x0 * v10 + rv0  (rv0 broadcast over the class dim)
            nc.vector.scalar_tensor_tensor(
                out=logits_sb[:, b, :].rearrange("p (j c) -> p j c", c=C),
                in0=x0_sb[:, b, :].rearrange("p (j c) -> p j c", c=C),
                scalar=v10[:, b : b + 1],
                in1=rv_b,
                op0=ALU.mult, op1=ALU.add,
            )
        nc.gpsimd.dma_start(
            out=logits_d.flatten().rearrange("(b p f) -> p b f", b=B_LOC, p=P, f=FREE),
            in_=logits_sb,
        )


def build_nc():
    nc = bacc.Bacc("TRN2", target_bir_lowering=False, debug=False, enable_asserts=False)
    aps = {
        "x0": nc.dram_tensor("x0", [B_LOC, S, C], F32, kind="ExternalInput").ap(),
        "cst": nc.dram_tensor("cst", [1, CST_N], F32, kind="ExternalInput").ap(),
        "logits": nc.dram_tensor(
            "logits", [B_LOC, S, C], F32, kind="ExternalOutput"
        ).ap(),
        "qt": nc.dram_tensor(
            "qt", [B_LOC, P, QT_REP, QT_F], F32, kind="ExternalOutput"
        ).ap(),
    }
    with tile.TileContext(nc) as tc:
        emit_kernel(tc, aps)
    nc.compile()
    return nc


_NC_CACHE = None


def _get_nc():
    global _NC_CACHE
    if _NC_CACHE is None:
        _NC_CACHE = build_nc()
    return _NC_CACHE


def make_in_maps(x_0: np.ndarray, t: np.ndarray) -> list:
    x_0 = np.ascontiguousarray(np.asarray(x_0), dtype=np.float32)
    t_f = np.asarray(t).astype(np.float32)
    tail = np.concatenate([np.arange(T_STEPS, dtype=np.float32), _alpha_table()])
    in_maps = []
    for c in range(N_CORES):
        lo = c * B_LOC
        cst = np.concatenate([t_f[lo : lo + B_LOC], tail]).reshape(1, CST_N)
        in_maps.append(
            {
                "x0": np.ascontiguousarray(x_0[lo : lo + B_LOC]),
                "cst": np.ascontiguousarray(cst),
            }
        )
    return in_maps


def assemble(results: list):
    logits = np.concatenate([r["logits"] for r in results], axis=0)
    q_t = np.concatenate(
        [r["qt"].reshape(B_LOC, S, C, C) for r in results], axis=0
    )
    return logits, q_t


def kernel(x_0, t):
    nc = _get_nc()
    res = run_bass_kernel_spmd(nc, make_in_maps(x_0, t), core_ids=list(range(N_CORES)))
    return assemble(res.results)


# revision 47
# speedup vs baseline: 1.2294x; 1.1064x over previous
"""Trainium2 Bass kernel for nn_DiscreteDiffusion (q_sample with logits).

Math: for each batch b with diffusion step t[b]:
    alpha = alpha_cumprod[t[b]]
    beta  = (1 - alpha) / C              (uniform-target mass per class)
    q_small[c,d] = alpha * I[c,d] + beta  (all off-diagonal entries equal beta,
                                           diagonal entries equal alpha + beta)
    log_q = log(q_small + EPS) has only TWO distinct values:
        v0 = log(beta + EPS)            (off-diagonal)
        v1 = log(alpha + beta + EPS)    (diagonal)
    =>  logits[s,d] = (v1 - v0) * x0[s,d] + v0 * sum_c x0[s,c]
    q_t[s] = q_small for every position s  (pure broadcast -> memory bound)

Sharding: pure data parallel over batch, 8 batches per core on 8 cores.
"""

import numpy as np
from contextlib import ExitStack

import concourse.bass as bass
import concourse.bacc as bacc
import concourse.tile as tile
from concourse import mybir
from concourse.bass_utils import run_bass_kernel_spmd

F32 = mybir.dt.float32
ALU = mybir.AluOpType
ACTF = mybir.ActivationFunctionType

T_STEPS = 200
C = 21
EPS = 1e-10
PRECISION = 1e-4

N_CORES = 8
B = 64
S = 4096
B_LOC = B // N_CORES          # 8 batches per core

P = 128                       # SBUF partitions
SPP = S // P                  # 32 sequence rows per partition (per batch)
FREE = SPP * C                # 672 floats per partition per batch
CC = C * C                    # 441
QT_REP = 4                    # DMA replication factor (step-0 source dim)
QT_F = (SPP // QT_REP) * CC   # floats per partition per qt tile
assert P * QT_REP * QT_F == S * CC
# single packed small-input row: t | iota | alpha table
# (one DMA so consuming DVE ops carry a single sync wait — the STT
#  instruction struct has only one wait slot)
CST_N = B_LOC + 2 * T_STEPS


def _alpha_table() -> np.ndarray:
    """fp32 replica of reference._alpha_cumprod (cos computed in f64, rounded)."""
    steps = np.arange(T_STEPS + 1, dtype=np.float32) / np.float32(T_STEPS)
    arg = (steps + np.float32(PRECISION)) / np.float32(1.0 + PRECISION)
    arg = arg * np.float32(np.pi) / np.float32(2.0)
    ac = np.cos(arg.astype(np.float64)).astype(np.float32) ** 2
    ac = ac / ac[0]
    betas = np.clip((np.float32(1.0) - ac[1:] / ac[:-1]), 0.0, 0.999).astype(np.float32)
    alphas = np.float32(1.0) - betas
    return np.cumprod(alphas, dtype=np.float32)  # [200]


def _bcast_p(ap: bass.AP, p: int = P) -> bass.AP:
    """[1, N] DRAM access pattern -> [p, N] partition-broadcast (step 0)."""
    return bass.AP(tensor=ap.tensor, offset=ap.offset, ap=[[0, p]] + list(ap.ap[1:]))


def emit_kernel(tc, aps: dict):
    """Emit the per-core program. aps maps names -> DRAM APs."""
    nc = tc.nc
    x0_d = aps["x0"]          # [B_LOC, S, C] f32 in
    cst_d = aps["cst"]        # [1, CST_N] f32 in (t | iota | table)
    logits_d = aps["logits"]  # [B_LOC, S, C] f32 out
    qt_d = aps["qt"]          # [B_LOC, P, QT_REP, QT_F] f32 out

    with ExitStack() as ctx:
        singles = ctx.enter_context(tc.tile_pool(name="singles", bufs=1))
        scratch = ctx.enter_context(tc.tile_pool(name="scratch", bufs=2))
        # One buffer per batch, never recycled: several instruction structs
        # here (TensorScalarPtr, direct DMA) have a single HW sync-wait
        # slot, so slot-reuse WAR waits must not exist; a deep pool also
        # lets all 8 q_t stores queue up front so the SDMA engines stay
        # saturated to the end.  The 8 q_t stores go on HWDGE (exactly its
        # 8 sem lanes, no lane-reuse waits); the other DMAs ride SWDGE.
        qtp = ctx.enter_context(tc.tile_pool(name="qtp", bufs=B_LOC))

        # ---- one broadcast DMA for every small input.  Emitted first so
        # the Q7 core preps these descriptors before its mask memset ------
        cst_sb = singles.tile([P, CST_N], F32)
        nc.gpsimd.dma_start(out=cst_sb, in_=_bcast_p(cst_d))
        o = B_LOC
        t_sb = cst_sb[:, 0:B_LOC]
        idx_sb = cst_sb[:, o : o + T_STEPS]
        tab_sb = cst_sb[:, o + T_STEPS : o + 2 * T_STEPS]

        # x0 load on SWDGE, prepped right after cst on the Q7 core; its
        # packets stay clear of the HWDGE q_t stream
        x0_sb = singles.tile([P, B_LOC, FREE], F32)
        nc.gpsimd.dma_start(
            out=x0_sb,
            in_=x0_d.flatten().rearrange("(b p f) -> p b f", b=B_LOC, p=P, f=FREE),
        )

        # diag mask built on device (no DMA), all on DVE so the whole q_t
        # chain is same-engine program order with no cost-model surprises
        n_rep_cc = QT_F // CC
        msk_sb = singles.tile([P, QT_F], F32)
        nc.vector.memset(msk_sb, 0.0)
        ma = msk_sb[:, :]
        m_diag = bass.AP(
            tensor=ma.tensor, offset=ma.offset,
            ap=[list(ma.ap[0]), [CC, n_rep_cc], [C + 1, C]],
        )
        nc.vector.memset(m_diag, 1.0)

        # ---- per-batch scalars, computed redundantly on all partitions ---
        alpha = singles.tile([P, B_LOC], F32)
        beta = singles.tile([P, B_LOC], F32)
        diag = singles.tile([P, B_LOC], F32)
        v0 = singles.tile([P, B_LOC], F32)
        v1 = singles.tile([P, B_LOC], F32)
        v10 = singles.tile([P, B_LOC], F32)

        for b in range(B_LOC):
            eq = scratch.tile([P, T_STEPS], F32, tag="eq")
            # eq = (idx == t[b]) * table ; alpha[b] = sum(eq) (fused accum)
            nc.vector.scalar_tensor_tensor(
                out=eq, in0=idx_sb, scalar=t_sb[:, b : b + 1], in1=tab_sb,
                op0=ALU.is_equal, op1=ALU.mult,
                accum_out=alpha[:, b : b + 1],
            )
        # beta = (1 - alpha)/C  computed as alpha*(-1/C) + 1/C
        nc.vector.tensor_scalar(
            out=beta, in0=alpha, scalar1=-1.0 / C, scalar2=1.0 / C,
            op0=ALU.mult, op1=ALU.add,
        )
        nc.vector.tensor_add(diag, alpha, beta)          # alpha + beta
        eps_sb = singles.tile([P, 1], F32)
        nc.vector.memset(eps_sb, EPS)
        nc.scalar.activation(v0, beta, ACTF.Ln, bias=eps_sb[:, 0:1])  # log(beta+eps)
        nc.scalar.activation(v1, diag, ACTF.Ln, bias=eps_sb[:, 0:1])  # log(diag+eps)
        nc.vector.tensor_sub(v10, v1, v0)

        # ---- q_t: build one [P, QT_F] pattern per batch (mask*alpha+beta),
        # fan out via a DMA with a step-0 repeat dim --------------------
        last_qt_ts = None
        for b in range(B_LOC):
            q = qtp.tile([P, QT_F], F32, tag="qt")
            last_qt_ts = nc.vector.tensor_scalar(
                out=q, in0=msk_sb,
                scalar1=alpha[:, b : b + 1], scalar2=beta[:, b : b + 1],
                op0=ALU.mult, op1=ALU.add,
            )
            qa = q[:, :]
            rep = bass.AP(
                tensor=qa.tensor, offset=qa.offset,
                ap=[list(qa.ap[0]), [0, QT_REP], list(qa.ap[1])],
            )
            nc.sync.dma_start(out=qt_d[b], in_=rep)

        # ---- logits ------------------------------------------------------
        r = singles.tile([P, B_LOC, SPP], F32)
        red = nc.vector.reduce_sum(
            out=r,
            in_=x0_sb[:, :, :].rearrange("p b (j c) -> p b j c", c=C),
            axis=mybir.AxisListType.X,
        )
        # the big reduce must not be scheduled into the DVE stream ahead of
        # the latency-critical q_t pattern builds
        tile.add_dep_helper(red.ins, last_qt_ts.ins, reason="qt ts first")
        rv0 = singles.tile([P, B_LOC, SPP], F32)
        logits_sb = singles.tile([P, B_LOC, FREE], F32)
        for b in range(B_LOC):
            nc.vector.tensor_scalar(
                out=rv0[:, b, :], in0=r[:, b, :],
                scalar1=v0[:, b : b + 1], scalar2=None, op0=ALU.mult,
            )
            rva = rv0[:, b, :]
            rv_b = bass.AP(
                tensor=rva.tensor, offset=rva.offset,
                ap=[list(rva.ap[0]), list(rva.ap[1]), [0, C]],
            )
            # logits = x0 * v10 + rv0  (rv0 broadcast over the class dim)
            nc.vector.scalar_tensor_tensor(
                out=logits_sb[:, b, :].rearrange("p (j c) -> p j c", c=C),
                in0=x0_sb[:, b, :].rearrange("p (j c) -> p j c", c=C),
                scalar=v10[:, b : b + 1],
                in1=rv_b,
                op0=ALU.mult, op1=ALU.add,
            )
        nc.gpsimd.dma_start(
            out=logits_d.flatten().rearrange("(b p f) -> p b f", b=B_LOC, p=P, f=FREE),
            in_=logits_sb,
        )


def build_nc():
    nc = bacc.Bacc("TRN2", target_bir_lowering=False, debug=False, enable_asserts=False)
    aps = {
        "x0": nc.dram_tensor("x0", [B_LOC, S, C], F32, kind="ExternalInput").ap(),
        "cst": nc.dram_tensor("cst", [1, CST_N], F32, kind="ExternalInput").ap(),
        "logits": nc.dram_tensor(
            "logits", [B_LOC, S, C], F32, kind="ExternalOutput"
        ).ap(),
        "qt": nc.dram_tensor(
            "qt", [B_LOC, P, QT_REP, QT_F], F32, kind="ExternalOutput"
        ).ap(),
    }
    with tile.TileContext(nc) as tc:
        emit_kernel(tc, aps)
    nc.compile()
    return nc


_NC_CACHE = None


def _get_nc():
    global _NC_CACHE
    if _NC_CACHE is None:
        _NC_CACHE = build_nc()
    return _NC_CACHE


def make_in_maps(x_0: np.ndarray, t: np.ndarray) -> list:
    x_0 = np.ascontiguousarray(np.asarray(x_0), dtype=np.float32)
    t_f = np.asarray(t).astype(np.float32)
    tail = np.concatenate([np.arange(T_STEPS, dtype=np.float32), _alpha_table()])
    in_maps = []
    for c in range(N_CORES):
        lo = c * B_LOC
        cst = np.concatenate([t_f[lo : lo + B_LOC], tail]).reshape(1, CST_N)
        in_maps.append(
            {
                "x0": np.ascontiguousarray(x_0[lo : lo + B_LOC]),
                "cst": np.ascontiguousarray(cst),
            }
        )
    return in_maps


def assemble(results: list):
    logits = np.concatenate([r["logits"] for r in results], axis=0)
    q_t = np.concatenate(
        [r["qt"].reshape(B_LOC, S, C, C) for r in results], axis=0
    )
    return logits, q_t


def kernel(x_0, t):
    nc = _get_nc()
    res = run_bass_kernel_spmd(nc, make_in_maps(x_0, t), core_ids=list(range(N_CORES)))
    return assemble(res.results)
